# revision 5
# baseline (speedup 1.0000x reference)
"""CenterLoss kernel for Trainium2 (8 NeuronCores, data-parallel over batch).

reference: mean(clip(distmat[i, labels[i]])) where
  distmat[i,c] = ||x_i||^2 + ||c_c||^2 - 2 x_i . c_c
i.e. the loss only needs dist_i = ||x_i - centers[labels[i]]||^2 — a gather +
elementwise + reduce; the full (N, C) matmul in the reference is dead work.

Per core (512 rows of the 4096-row batch):
  - x shard enters SBUF as one contiguous [128, 4*512] tile (partition p holds
    rows 4p..4p+3).
  - labels shard enters as [128, 4] int32; column j drives an indirect-DMA
    gather of centers rows into a [128, 512] tile.
  - VectorE: diff = x - c;  ScalarE: Square activation with accum_out giving
    the per-row sum of squares directly.
  - [128, 4] per-row distances DMA out; host concatenates 8 cores, applies the
    clip (a no-op for this data but kept for exactness) and the mean.
"""

import numpy as np

N, D, C = 4096, 512, 10000
NCORES = 8
ROWS_PER_CORE = N // NCORES  # 512
P = 128
J = ROWS_PER_CORE // P  # 4 rows per partition

CLAMP = 1e-12

_cache = {}


def _build_nc():
    import concourse.bass as bass
    import concourse.mybir as mybir
    from concourse import bacc
    from concourse.tile import TileContext

    nc = bacc.Bacc(
        "TRN2", target_bir_lowering=False, debug=False, num_devices=NCORES
    )
    x = nc.dram_tensor("x", [P, J * D], mybir.dt.float32, kind="ExternalInput")
    labels = nc.dram_tensor("labels", [P, J], mybir.dt.int32, kind="ExternalInput")
    centers = nc.dram_tensor("centers", [C, D], mybir.dt.float32, kind="ExternalInput")
    out = nc.dram_tensor("out", [P, J], mybir.dt.float32, kind="ExternalOutput")

    with TileContext(nc) as tc:
        with (
            tc.tile_pool(name="io", bufs=1) as io_pool,
            tc.tile_pool(name="work", bufs=J) as work,
        ):
            # labels first on the SP HW ring — the gathers are gated on it
            lab_tile = io_pool.tile([P, J], mybir.dt.int32)
            nc.sync.dma_start(out=lab_tile[:], in_=labels[:])

            # all 4 gather instructions right behind labels on the Pool queue
            ctiles = []
            for j in range(J):
                ctile = work.tile([P, D], mybir.dt.float32, tag="c")
                nc.gpsimd.indirect_dma_start(
                    out=ctile[:],
                    out_offset=None,
                    in_=centers[:],
                    in_offset=bass.IndirectOffsetOnAxis(
                        ap=lab_tile[:, j : j + 1], axis=0
                    ),
                )
                ctiles.append(ctile)

            # x chunks on the Act HW ring (keeps SP ring free for labels/out)
            x_tile = io_pool.tile([P, J * D], mybir.dt.float32)
            for j in range(J):
                nc.scalar.dma_start(
                    out=x_tile[:, j * D : (j + 1) * D],
                    in_=x[:, j * D : (j + 1) * D],
                )

            dists = io_pool.tile([P, J], mybir.dt.float32)

            for j in range(J):
                diff = work.tile([P, D], mybir.dt.float32, tag="d")
                nc.vector.tensor_tensor(
                    out=diff[:],
                    in0=x_tile[:, j * D : (j + 1) * D],
                    in1=ctiles[j][:],
                    op=mybir.AluOpType.subtract,
                )
                sq = work.tile([P, D], mybir.dt.float32, tag="s")
                nc.scalar.activation(
                    out=sq[:],
                    in_=diff[:],
                    func=mybir.ActivationFunctionType.Square,
                    accum_out=dists[:, j : j + 1],
                )

            nc.sync.dma_start(out=out[:], in_=dists[:])

    nc.compile()
    return nc


def _run(in_maps, trace=False):
    from concourse.bass_utils import run_bass_kernel_spmd

    if "nc" not in _cache:
        _cache["nc"] = _build_nc()
    return run_bass_kernel_spmd(
        _cache["nc"], in_maps, list(range(NCORES)), trace=trace
    )


def kernel(x, labels, centers, _trace=False):
    x = np.ascontiguousarray(np.asarray(x, dtype=np.float32))
    labels = np.asarray(labels).astype(np.int32)
    centers = np.ascontiguousarray(np.asarray(centers, dtype=np.float32))

    in_maps = []
    for c in range(NCORES):
        lo = c * ROWS_PER_CORE
        hi = lo + ROWS_PER_CORE
        in_maps.append(
            {
                "x": x[lo:hi].reshape(P, J * D),
                "labels": np.ascontiguousarray(labels[lo:hi].reshape(P, J)),
                "centers": centers,
            }
        )

    res = _run(in_maps, trace=_trace)
    dists = np.concatenate(
        [res.results[c]["out"].reshape(ROWS_PER_CORE) for c in range(NCORES)]
    )
    loss = np.clip(dists, CLAMP, 1.0 / CLAMP).mean(dtype=np.float64)
    out = np.asarray(loss, dtype=np.float32)
    if _trace:
        return out, res
    return out
